# revision 3
# baseline (speedup 1.0000x reference)
"""Chamfer-distance kernel for Trainium2 (nn_CD_1013612282415).

Full inputs: pred [8, 8192, 3] f32, gt [8, 8192, 3] f32.
Output: scalar f32 = mean_b(0.5*mean_n min_m ||p-g||^2 + 0.5*mean_m min_n) * 100.

Sharding: one batch element per NeuronCore (8 cores).

Per-core algorithm:
  The squared-distance matrix is computed on the PE as a single K=13 fp16
  matmul per tile: each operand value is hi/lo-split into two fp16s and the
  product u*v expanded as uh*vh + uh*vl + ul*vh across K-rows (K-rows are
  free: matmul cost is free-dim cycles only). This gives ~1e-5 abs accuracy
  (vs ~7e-3 for a plain fp16/bf16 matmul) at full bf16 streaming rate --
  4x faster than the native fp32 matmul path.

  dis tiles land in PSUM [128, 2048] f32. ScalarE casts them to bf16 in
  SBUF; VectorE then does the row-min via tensor_scalar's min-accumulator
  (single-source op -> 4x DVE mode) and folds the running col-min via
  bf16 tensor_tensor min (2x mode). The col-min partition reduction runs
  at the end via PE transposes + strided tensor_reduce. Per-core output is
  [sum_n rowmin, sum_m colmin]; the host combines 8 pairs in float64.
"""
import os
import sys

for _p in ("/opt/trn_rl_repo",):
    if _p not in sys.path:
        sys.path.insert(0, _p)

import numpy as np
import concourse.bass as bass
import concourse.mybir as mybir
from concourse.tile import TileContext
from concourse.bass_utils import run_bass_kernel_spmd

B, N, M, D = 8, 8192, 8192, 3
K = 13            # 3 coord dims x 3 split rows + 2 (|p|^2) + 2 (|g|^2)
PCHUNK = 128      # n rows per matmul tile (partition dim)
FD = 2048         # m columns per PSUM supertile (4 banks)
NI = N // PCHUNK  # 64 n-chunks
NJ = M // FD      # 4 m-superchunks
MM_N = 512        # columns per matmul (one PSUM bank)
BIG = 3.0e38

_CORES = list(range(8))
_NC_CACHE = {}
LAST_PROFILE = {}


def _split_waits(nc, max_waits=1):
    """This container's pinned walrus rejects >1 sync-wait per instruction;
    move excess waits onto InstNoOps inserted just before the offender."""
    for f in nc.m.functions:
        for bb in f.blocks:
            insts = list(bb.instructions)
            out, changed = [], False
            for inst in insts:
                si = inst.sync_info
                if si is not None and len(si.on_wait) > max_waits:
                    waits = list(si.on_wait)
                    extra, keep = waits[:-max_waits], waits[-max_waits:]
                    for i in range(0, len(extra), max_waits):
                        nop = mybir.InstNoOp(
                            name=f"{inst.name}-wsplit-{i}",
                            sync_info=mybir.SyncInfo(
                                on_wait=extra[i : i + max_waits], on_update=[]
                            ),
                        )
                        nop.engine = inst.engine
                        out.append(nop)
                    inst.sync_info = mybir.SyncInfo(
                        on_wait=keep, on_update=list(si.on_update)
                    )
                    changed = True
                out.append(inst)
            if changed:
                bb.instructions = out


def _build_nc():
    f16, f32, bf16, i32 = (
        mybir.dt.float16,
        mybir.dt.float32,
        mybir.dt.bfloat16,
        mybir.dt.int32,
    )
    nc = bass.Bass(trn_type="TRN2")
    a_dram = nc.declare_dram_parameter("a", [K, N], f16, isOutput=False)
    b_dram = nc.declare_dram_parameter("b", [K, M], f16, isOutput=False)
    out_dram = nc.declare_dram_parameter("out", [1, 2], f32, isOutput=True)

    with TileContext(nc) as tc:
        with (
            tc.tile_pool(name="io", bufs=1) as io,
            tc.tile_pool(name="work", bufs=1) as work,
            tc.tile_pool(name="dis", bufs=4) as disp,
            tc.tile_pool(name="rowt", bufs=4) as rowt,
        ):
            a_sb = io.tile([K, N], f16)
            b_sb = io.tile([K, M], f16)
            nc.sync.dma_start(out=a_sb[:], in_=a_dram.ap())
            nc.sync.dma_start(out=b_sb[:], in_=b_dram.ap())

            colmin = [work.tile([PCHUNK, FD], bf16, name=f"colmin{j}") for j in range(NJ)]
            for j in range(NJ):
                nc.vector.memset(colmin[j][:], BIG)
            rowmins = work.tile([PCHUNK, NI], f32)

            # identity (bf16) for PE transposes, built on device
            col_i = work.tile([PCHUNK, PCHUNK], i32)
            part_i = work.tile([PCHUNK, PCHUNK], i32)
            nc.gpsimd.iota(col_i[:], pattern=[[1, PCHUNK]], channel_multiplier=0)
            nc.gpsimd.iota(part_i[:], pattern=[[0, PCHUNK]], channel_multiplier=1)
            ident = work.tile([PCHUNK, PCHUNK], bf16)
            nc.vector.tensor_tensor(
                ident[:], col_i[:], part_i[:], mybir.AluOpType.is_equal
            )

            with tc.tile_pool(name="ps", bufs=2, space="PSUM") as ps:
                for i in range(NI):
                    lhsT = a_sb[:, i * PCHUNK : (i + 1) * PCHUNK]
                    rtmp = rowt.tile([PCHUNK, NJ], f32, name="rtmp")
                    for j in range(NJ):
                        psum = ps.tile([PCHUNK, FD], f32, name="psum")
                        for s in range(FD // MM_N):
                            c0 = j * FD + s * MM_N
                            nc.tensor.matmul(
                                psum[:, s * MM_N : (s + 1) * MM_N],
                                lhsT,
                                b_sb[:, c0 : c0 + MM_N],
                                start=True,
                                stop=True,
                            )
                        dis_bf = disp.tile([PCHUNK, FD], bf16, name="dis_bf")
                        nc.scalar.copy(dis_bf[:], psum[:])
                        # row-min of this tile -> rtmp[:, j] (4x DVE mode)
                        nc.vector.tensor_scalar(
                            dis_bf[:],
                            dis_bf[:],
                            BIG,
                            None,
                            mybir.AluOpType.min,
                            mybir.AluOpType.min,
                            accum_out=rtmp[:, j : j + 1],
                        )
                        # fold running col-min (2x DVE mode)
                        nc.vector.tensor_tensor(
                            colmin[j][:], dis_bf[:], colmin[j][:], mybir.AluOpType.min
                        )
                    nc.vector.tensor_reduce(
                        rowmins[:, i : i + 1],
                        rtmp[:],
                        mybir.AxisListType.X,
                        mybir.AluOpType.min,
                    )

            # epilogue: col-min partition reduction via PE transposes
            sums = work.tile([PCHUNK, 2], f32)
            cmin_t = work.tile([PCHUNK, NJ * (FD // PCHUNK)], f32, name="cmin_t")
            with tc.tile_pool(name="pst", bufs=2, space="PSUM") as pst:
                for j in range(NJ):
                    tp = pst.tile([PCHUNK, FD], bf16, name="tp")
                    for k in range(FD // PCHUNK):
                        nc.tensor.transpose(
                            tp[:, k * PCHUNK : (k + 1) * PCHUNK],
                            colmin[j][:, k * PCHUNK : (k + 1) * PCHUNK],
                            ident[:],
                        )
                    nb = FD // PCHUNK  # 16 blocks
                    nc.vector.tensor_reduce(
                        cmin_t[:, j * nb : (j + 1) * nb],
                        tp[:].rearrange("p (k q) -> p k q", q=PCHUNK),
                        mybir.AxisListType.X,
                        mybir.AluOpType.min,
                    )
                nc.vector.tensor_reduce(
                    sums[:, 0:1], rowmins[:], mybir.AxisListType.X, mybir.AluOpType.add
                )
                nc.vector.tensor_reduce(
                    sums[:, 1:2], cmin_t[:], mybir.AxisListType.X, mybir.AluOpType.add
                )
                ones = work.tile([PCHUNK, 1], f32)
                nc.vector.memset(ones[:], 1.0)
                out_ps = pst.tile([1, 2], f32, name="out_ps")
                nc.tensor.matmul(out_ps[:], ones[:], sums[:], start=True, stop=True)
                out_sb = work.tile([1, 2], f32)
                nc.scalar.copy(out_sb[:], out_ps[:])
                nc.sync.dma_start(out=out_dram.ap(), in_=out_sb[:])

    _split_waits(nc)
    return nc


def _split16(x):
    hi = x.astype(np.float16)
    lo = (x.astype(np.float32) - hi.astype(np.float32)).astype(np.float16)
    return hi, lo


def _make_aug(p, g):
    """p [N,3] f32, g [M,3] f32 -> A [13, N] f16, B [13, M] f16 such that
    (A.T @ B)[n, m] ~= ||p_n - g_m||^2 to ~1e-5."""
    u = (-2.0 * p.T).astype(np.float32)          # [3, N]
    v = np.ascontiguousarray(g.T)                # [3, M]
    p2 = (p * p).sum(1, dtype=np.float32)
    g2 = (g * g).sum(1, dtype=np.float32)
    uh, ul = _split16(u)
    vh, vl = _split16(v)
    p2h, p2l = _split16(p2)
    g2h, g2l = _split16(g2)
    onesN = np.ones(p.shape[0], np.float16)
    onesM = np.ones(g.shape[0], np.float16)
    A_rows, B_rows = [], []
    for d in range(D):
        A_rows += [uh[d], uh[d], ul[d]]
        B_rows += [vh[d], vl[d], vh[d]]
    A_rows += [p2h, p2l, onesN, onesN]
    B_rows += [onesM, onesM, g2h, g2l]
    return np.stack(A_rows), np.stack(B_rows)


def kernel(pred: np.ndarray, gt: np.ndarray) -> np.ndarray:
    pred = np.asarray(pred, dtype=np.float32)
    gt = np.asarray(gt, dtype=np.float32)
    assert pred.shape == (B, N, D) and gt.shape == (B, M, D)

    in_maps = []
    for b in range(B):
        A, Bm = _make_aug(pred[b], gt[b])
        in_maps.append({"a": A, "b": Bm})

    if "nc" not in _NC_CACHE:
        _NC_CACHE["nc"] = _build_nc()
    nc = _NC_CACHE["nc"]

    trace = bool(int(os.environ.get("KERNEL_TRACE", "0")))
    res = run_bass_kernel_spmd(nc, in_maps, _CORES, trace=trace)
    LAST_PROFILE.clear()
    LAST_PROFILE.update(
        exec_time_ns=res.exec_time_ns, mean_exec_time_ns=res.mean_exec_time_ns
    )
    if trace and res.instructions_and_trace is not None:
        LAST_PROFILE["trace_path"] = res.instructions_and_trace[1]

    total = 0.0
    for b in range(B):
        rs, cs = (float(x) for x in res.results[b]["out"][0])
        total += 0.5 * (rs / N + cs / M)
    return np.float32(total / B * 100.0)


# revision 4
# speedup vs baseline: 1.2823x; 1.2823x over previous
"""Chamfer-distance kernel for Trainium2 (nn_CD_1013612282415).

Full inputs: pred [8, 8192, 3] f32, gt [8, 8192, 3] f32.
Output: scalar f32 = mean_b(0.5*mean_n min_m ||p-g||^2 + 0.5*mean_m min_n) * 100.

Sharding: one batch element per NeuronCore (8 cores).

Per-core algorithm:
  The squared-distance matrix is computed on the PE as a single K=13 fp16
  matmul per tile: each operand value is hi/lo-split into two fp16s and the
  product u*v expanded as uh*vh + uh*vl + ul*vh across K-rows (K-rows are
  free: matmul cost is free-dim cycles only). This gives ~1e-5 abs accuracy
  (vs ~7e-3 for a plain fp16/bf16 matmul) at full bf16 streaming rate --
  4x faster than the native fp32 matmul path.

  dis tiles land in PSUM [128, 2048] f32. ScalarE casts them to bf16 in
  SBUF; VectorE then does the row-min via tensor_scalar's min-accumulator
  (single-source op -> 4x DVE mode) and folds the running col-min via
  bf16 tensor_tensor min (2x mode). The col-min partition reduction runs
  at the end via PE transposes + strided tensor_reduce. Per-core output is
  [sum_n rowmin, sum_m colmin]; the host combines 8 pairs in float64.
"""
import os
import sys

for _p in ("/opt/trn_rl_repo",):
    if _p not in sys.path:
        sys.path.insert(0, _p)

import numpy as np
import concourse.bass as bass
import concourse.mybir as mybir
from concourse.tile import TileContext
from concourse.bass_utils import run_bass_kernel_spmd

B, N, M, D = 8, 8192, 8192, 3
K = 13            # 3 coord dims x 3 split rows + 2 (|p|^2) + 2 (|g|^2)
PCHUNK = 128      # n rows per matmul tile (partition dim)
FD = 2048         # m columns per PSUM supertile (4 banks)
NI = N // PCHUNK  # 64 n-chunks
NJ = M // FD      # 4 m-superchunks
MM_N = 512        # columns per matmul (one PSUM bank)
BIG = 3.0e38

_CORES = list(range(8))
_NC_CACHE = {}
LAST_PROFILE = {}


def _split_waits(nc, max_waits=1):
    """This container's pinned walrus rejects >1 sync-wait per instruction;
    move excess waits onto InstNoOps inserted just before the offender."""
    for f in nc.m.functions:
        for bb in f.blocks:
            insts = list(bb.instructions)
            out, changed = [], False
            for inst in insts:
                si = inst.sync_info
                if si is not None and len(si.on_wait) > max_waits:
                    waits = list(si.on_wait)
                    extra, keep = waits[:-max_waits], waits[-max_waits:]
                    for i in range(0, len(extra), max_waits):
                        nop = mybir.InstNoOp(
                            name=f"{inst.name}-wsplit-{i}",
                            sync_info=mybir.SyncInfo(
                                on_wait=extra[i : i + max_waits], on_update=[]
                            ),
                        )
                        nop.engine = inst.engine
                        out.append(nop)
                    inst.sync_info = mybir.SyncInfo(
                        on_wait=keep, on_update=list(si.on_update)
                    )
                    changed = True
                out.append(inst)
            if changed:
                bb.instructions = out


def _build_nc():
    f16, f32, bf16, i32 = (
        mybir.dt.float16,
        mybir.dt.float32,
        mybir.dt.bfloat16,
        mybir.dt.int32,
    )
    nc = bass.Bass(trn_type="TRN2")
    a_dram = nc.declare_dram_parameter("a", [K, N], f16, isOutput=False)
    b_dram = nc.declare_dram_parameter("b", [K, M], f16, isOutput=False)
    out_dram = nc.declare_dram_parameter("out", [1, 2], f32, isOutput=True)

    with TileContext(nc) as tc:
        with (
            tc.tile_pool(name="io", bufs=1) as io,
            tc.tile_pool(name="work", bufs=1) as work,
            tc.tile_pool(name="dis", bufs=4) as disp,
            tc.tile_pool(name="rowt", bufs=4) as rowt,
        ):
            a_sb = io.tile([K, N], f16)
            b_sb = io.tile([K, M], f16)
            nc.sync.dma_start(out=a_sb[:], in_=a_dram.ap())
            nc.sync.dma_start(out=b_sb[:], in_=b_dram.ap())

            colmin = [work.tile([PCHUNK, FD], bf16, name=f"colmin{j}") for j in range(NJ)]
            for j in range(NJ):
                nc.vector.memset(colmin[j][:], BIG)
            rowmins = work.tile([PCHUNK, NI], f32)

            # identity (bf16) for PE transposes, built on device
            col_i = work.tile([PCHUNK, PCHUNK], i32)
            part_i = work.tile([PCHUNK, PCHUNK], i32)
            nc.gpsimd.iota(col_i[:], pattern=[[1, PCHUNK]], channel_multiplier=0)
            nc.gpsimd.iota(part_i[:], pattern=[[0, PCHUNK]], channel_multiplier=1)
            ident = work.tile([PCHUNK, PCHUNK], bf16)
            nc.vector.tensor_tensor(
                ident[:], col_i[:], part_i[:], mybir.AluOpType.is_equal
            )

            with tc.tile_pool(name="ps", bufs=2, space="PSUM") as ps:
                for i in range(NI):
                    lhsT = a_sb[:, i * PCHUNK : (i + 1) * PCHUNK]
                    rowacc = rowt.tile([PCHUNK, FD], bf16, name="rowacc", bufs=2)
                    for j in range(NJ):
                        psum = ps.tile([PCHUNK, FD], f32, name="psum")
                        for s in range(FD // MM_N):
                            c0 = j * FD + s * MM_N
                            nc.tensor.matmul(
                                psum[:, s * MM_N : (s + 1) * MM_N],
                                lhsT,
                                b_sb[:, c0 : c0 + MM_N],
                                start=True,
                                stop=True,
                            )
                        dis_bf = disp.tile([PCHUNK, FD], bf16, name="dis_bf")
                        nc.scalar.copy(dis_bf[:], psum[:])
                        # row-direction elementwise fold (copy 4x / TT-min 2x)
                        if j == 0:
                            nc.vector.tensor_copy(rowacc[:], dis_bf[:])
                        else:
                            nc.vector.tensor_tensor(
                                rowacc[:], dis_bf[:], rowacc[:], mybir.AluOpType.min
                            )
                        # fold running col-min (2x DVE mode)
                        nc.vector.tensor_tensor(
                            colmin[j][:], dis_bf[:], colmin[j][:], mybir.AluOpType.min
                        )
                    nc.vector.tensor_reduce(
                        rowmins[:, i : i + 1],
                        rowacc[:],
                        mybir.AxisListType.X,
                        mybir.AluOpType.min,
                    )

            # epilogue: col-min partition reduction via PE transposes
            sums = work.tile([PCHUNK, 2], f32)
            cmin_t = work.tile([PCHUNK, NJ * (FD // PCHUNK)], f32, name="cmin_t")
            with tc.tile_pool(name="pst", bufs=2, space="PSUM") as pst:
                for j in range(NJ):
                    tp = pst.tile([PCHUNK, FD], bf16, name="tp")
                    for k in range(FD // PCHUNK):
                        nc.tensor.transpose(
                            tp[:, k * PCHUNK : (k + 1) * PCHUNK],
                            colmin[j][:, k * PCHUNK : (k + 1) * PCHUNK],
                            ident[:],
                        )
                    nb = FD // PCHUNK  # 16 blocks
                    nc.vector.tensor_reduce(
                        cmin_t[:, j * nb : (j + 1) * nb],
                        tp[:].rearrange("p (k q) -> p k q", q=PCHUNK),
                        mybir.AxisListType.X,
                        mybir.AluOpType.min,
                    )
                nc.vector.tensor_reduce(
                    sums[:, 0:1], rowmins[:], mybir.AxisListType.X, mybir.AluOpType.add
                )
                nc.vector.tensor_reduce(
                    sums[:, 1:2], cmin_t[:], mybir.AxisListType.X, mybir.AluOpType.add
                )
                ones = work.tile([PCHUNK, 1], f32)
                nc.vector.memset(ones[:], 1.0)
                out_ps = pst.tile([1, 2], f32, name="out_ps")
                nc.tensor.matmul(out_ps[:], ones[:], sums[:], start=True, stop=True)
                out_sb = work.tile([1, 2], f32)
                nc.scalar.copy(out_sb[:], out_ps[:])
                nc.sync.dma_start(out=out_dram.ap(), in_=out_sb[:])

    _split_waits(nc)
    return nc


def _split16(x):
    hi = x.astype(np.float16)
    lo = (x.astype(np.float32) - hi.astype(np.float32)).astype(np.float16)
    return hi, lo


def _make_aug(p, g):
    """p [N,3] f32, g [M,3] f32 -> A [13, N] f16, B [13, M] f16 such that
    (A.T @ B)[n, m] ~= ||p_n - g_m||^2 to ~1e-5."""
    u = (-2.0 * p.T).astype(np.float32)          # [3, N]
    v = np.ascontiguousarray(g.T)                # [3, M]
    p2 = (p * p).sum(1, dtype=np.float32)
    g2 = (g * g).sum(1, dtype=np.float32)
    uh, ul = _split16(u)
    vh, vl = _split16(v)
    p2h, p2l = _split16(p2)
    g2h, g2l = _split16(g2)
    onesN = np.ones(p.shape[0], np.float16)
    onesM = np.ones(g.shape[0], np.float16)
    A_rows, B_rows = [], []
    for d in range(D):
        A_rows += [uh[d], uh[d], ul[d]]
        B_rows += [vh[d], vl[d], vh[d]]
    A_rows += [p2h, p2l, onesN, onesN]
    B_rows += [onesM, onesM, g2h, g2l]
    return np.stack(A_rows), np.stack(B_rows)


def kernel(pred: np.ndarray, gt: np.ndarray) -> np.ndarray:
    pred = np.asarray(pred, dtype=np.float32)
    gt = np.asarray(gt, dtype=np.float32)
    assert pred.shape == (B, N, D) and gt.shape == (B, M, D)

    in_maps = []
    for b in range(B):
        A, Bm = _make_aug(pred[b], gt[b])
        in_maps.append({"a": A, "b": Bm})

    if "nc" not in _NC_CACHE:
        _NC_CACHE["nc"] = _build_nc()
    nc = _NC_CACHE["nc"]

    trace = bool(int(os.environ.get("KERNEL_TRACE", "0")))
    res = run_bass_kernel_spmd(nc, in_maps, _CORES, trace=trace)
    LAST_PROFILE.clear()
    LAST_PROFILE.update(
        exec_time_ns=res.exec_time_ns, mean_exec_time_ns=res.mean_exec_time_ns
    )
    if trace and res.instructions_and_trace is not None:
        LAST_PROFILE["trace_path"] = res.instructions_and_trace[1]

    total = 0.0
    for b in range(B):
        rs, cs = (float(x) for x in res.results[b]["out"][0])
        total += 0.5 * (rs / N + cs / M)
    return np.float32(total / B * 100.0)


# revision 7
# speedup vs baseline: 1.4430x; 1.1253x over previous
"""Chamfer-distance kernel for Trainium2 (nn_CD_1013612282415).

Full inputs: pred [8, 8192, 3] f32, gt [8, 8192, 3] f32.
Output: scalar f32 = mean_b(0.5*mean_n min_m ||p-g||^2 + 0.5*mean_m min_n) * 100.

Sharding: one batch element per NeuronCore (8 cores).

Per-core algorithm:
  The squared-distance matrix is computed on the PE as a single K=13 fp16
  matmul per tile: each operand value is hi/lo-split into two fp16s and the
  product u*v expanded as uh*vh + uh*vl + ul*vh across K-rows (K-rows are
  free: matmul cost is free-dim cycles only). This gives ~1e-5 abs accuracy
  (vs ~7e-3 for a plain fp16/bf16 matmul) at full bf16 streaming rate --
  4x faster than the native fp32 matmul path.

  dis tiles land in PSUM [128, 2048] f32. ScalarE casts them to bf16 in
  SBUF; VectorE then does the row-min via tensor_scalar's min-accumulator
  (single-source op -> 4x DVE mode) and folds the running col-min via
  bf16 tensor_tensor min (2x mode). The col-min partition reduction runs
  at the end via PE transposes + strided tensor_reduce. Per-core output is
  [sum_n rowmin, sum_m colmin]; the host combines 8 pairs in float64.
"""
import os
import sys

for _p in ("/opt/trn_rl_repo",):
    if _p not in sys.path:
        sys.path.insert(0, _p)

import numpy as np
import concourse.bass as bass
import concourse.mybir as mybir
from concourse.tile import TileContext
from concourse.bass_utils import run_bass_kernel_spmd

B, N, M, D = 8, 8192, 8192, 3
K = 13            # 3 coord dims x 3 split rows + 2 (|p|^2) + 2 (|g|^2)
PCHUNK = 128      # n rows per matmul tile (partition dim)
FD = 2048         # m columns per PSUM supertile (4 banks)
NI = N // PCHUNK  # 64 n-chunks
NJ = M // FD      # 4 m-superchunks
MM_N = 512        # columns per matmul (one PSUM bank)
BIG = 3.0e38

_CORES = list(range(8))
_NC_CACHE = {}
LAST_PROFILE = {}


def _split_waits(nc, max_waits=1):
    """This container's pinned walrus rejects >1 sync-wait per instruction;
    move excess waits onto InstNoOps inserted just before the offender."""
    for f in nc.m.functions:
        for bb in f.blocks:
            insts = list(bb.instructions)
            out, changed = [], False
            for inst in insts:
                si = inst.sync_info
                if si is not None and len(si.on_wait) > max_waits:
                    waits = list(si.on_wait)
                    extra, keep = waits[:-max_waits], waits[-max_waits:]
                    for i in range(0, len(extra), max_waits):
                        nop = mybir.InstNoOp(
                            name=f"{inst.name}-wsplit-{i}",
                            sync_info=mybir.SyncInfo(
                                on_wait=extra[i : i + max_waits], on_update=[]
                            ),
                        )
                        nop.engine = inst.engine
                        out.append(nop)
                    inst.sync_info = mybir.SyncInfo(
                        on_wait=keep, on_update=list(si.on_update)
                    )
                    changed = True
                out.append(inst)
            if changed:
                bb.instructions = out


def _build_nc():
    f16, f32, bf16, i32 = (
        mybir.dt.float16,
        mybir.dt.float32,
        mybir.dt.bfloat16,
        mybir.dt.int32,
    )
    nc = bass.Bass(trn_type="TRN2")
    a_dram = nc.declare_dram_parameter("a", [K, N], f16, isOutput=False)
    b_dram = nc.declare_dram_parameter("b", [K, M], f16, isOutput=False)
    out_dram = nc.declare_dram_parameter("out", [1, 2], f32, isOutput=True)

    with TileContext(nc) as tc:
        with (
            tc.tile_pool(name="io", bufs=1) as io,
            tc.tile_pool(name="work", bufs=1) as work,
            tc.tile_pool(name="dis", bufs=4) as disp,
            tc.tile_pool(name="rowt", bufs=4) as rowt,
        ):
            a_sb = io.tile([K, N], f16)
            b_sb = io.tile([K, M], f16)
            nc.sync.dma_start(out=a_sb[:], in_=a_dram.ap())
            nc.sync.dma_start(out=b_sb[:], in_=b_dram.ap())

            colmin = work.tile([PCHUNK, M], bf16, name="colmin")
            nc.vector.memset(colmin[:], BIG)
            rowmins = work.tile([PCHUNK, NI], f32)

            # identity (bf16) for PE transposes, built on device
            col_i = work.tile([PCHUNK, PCHUNK], i32)
            part_i = work.tile([PCHUNK, PCHUNK], i32)
            nc.gpsimd.iota(col_i[:], pattern=[[1, PCHUNK]], channel_multiplier=0)
            nc.gpsimd.iota(part_i[:], pattern=[[0, PCHUNK]], channel_multiplier=1)
            ident = work.tile([PCHUNK, PCHUNK], bf16)
            nc.vector.tensor_tensor(
                ident[:], col_i[:], part_i[:], mybir.AluOpType.is_equal
            )

            with tc.tile_pool(name="ps", bufs=2, space="PSUM") as ps:
                for i in range(NI):
                    lhsT = a_sb[:, i * PCHUNK : (i + 1) * PCHUNK]
                    # contiguous bf16 row of all NJ supertiles for wide DVE ops
                    drow = disp.tile([PCHUNK, M], bf16, name="drow", bufs=2)
                    for j in range(NJ):
                        psum = ps.tile([PCHUNK, FD], f32, name="psum")
                        for s in range(FD // MM_N):
                            c0 = j * FD + s * MM_N
                            nc.tensor.matmul(
                                psum[:, s * MM_N : (s + 1) * MM_N],
                                lhsT,
                                b_sb[:, c0 : c0 + MM_N],
                                start=True,
                                stop=True,
                            )
                        nc.scalar.copy(drow[:, j * FD : (j + 1) * FD], psum[:])
                    # one wide col-min fold: visits 2x8192 inputs at 4/cyc
                    nc.vector.tensor_tensor(
                        colmin[:], drow[:], colmin[:], mybir.AluOpType.min
                    )
                    # row-min: pairwise halving tree, then one 1x reduce
                    t1 = rowt.tile([PCHUNK, M // 2], bf16, name="t1", bufs=2)
                    nc.vector.tensor_tensor(
                        t1[:], drow[:, : M // 2], drow[:, M // 2 :], mybir.AluOpType.min
                    )
                    nc.vector.tensor_tensor(
                        t1[:, : M // 4], t1[:, : M // 4], t1[:, M // 4 :], mybir.AluOpType.min
                    )
                    nc.vector.tensor_tensor(
                        t1[:, : M // 8], t1[:, : M // 8], t1[:, M // 8 : M // 4], mybir.AluOpType.min
                    )
                    nc.vector.tensor_reduce(
                        rowmins[:, i : i + 1],
                        t1[:, : M // 8],
                        mybir.AxisListType.X,
                        mybir.AluOpType.min,
                    )

            # epilogue: col-min partition reduction via PE transposes
            sums = work.tile([PCHUNK, 2], f32)
            cmin_t = work.tile([PCHUNK, NJ * (FD // PCHUNK)], f32, name="cmin_t")
            with tc.tile_pool(name="pst", bufs=2, space="PSUM") as pst:
                for j in range(NJ):
                    tp = pst.tile([PCHUNK, FD], bf16, name="tp")
                    for k in range(FD // PCHUNK):
                        c0 = j * FD + k * PCHUNK
                        nc.tensor.transpose(
                            tp[:, k * PCHUNK : (k + 1) * PCHUNK],
                            colmin[:, c0 : c0 + PCHUNK],
                            ident[:],
                        )
                    nb = FD // PCHUNK  # 16 blocks
                    nc.vector.tensor_reduce(
                        cmin_t[:, j * nb : (j + 1) * nb],
                        tp[:].rearrange("p (k q) -> p k q", q=PCHUNK),
                        mybir.AxisListType.X,
                        mybir.AluOpType.min,
                    )
                nc.vector.tensor_reduce(
                    sums[:, 0:1], rowmins[:], mybir.AxisListType.X, mybir.AluOpType.add
                )
                nc.vector.tensor_reduce(
                    sums[:, 1:2], cmin_t[:], mybir.AxisListType.X, mybir.AluOpType.add
                )
                ones = work.tile([PCHUNK, 1], f32)
                nc.vector.memset(ones[:], 1.0)
                out_ps = pst.tile([1, 2], f32, name="out_ps")
                nc.tensor.matmul(out_ps[:], ones[:], sums[:], start=True, stop=True)
                out_sb = work.tile([1, 2], f32)
                nc.scalar.copy(out_sb[:], out_ps[:])
                nc.sync.dma_start(out=out_dram.ap(), in_=out_sb[:])

    _split_waits(nc)
    return nc


def _split16(x):
    hi = x.astype(np.float16)
    lo = (x.astype(np.float32) - hi.astype(np.float32)).astype(np.float16)
    return hi, lo


def _make_aug(p, g):
    """p [N,3] f32, g [M,3] f32 -> A [13, N] f16, B [13, M] f16 such that
    (A.T @ B)[n, m] ~= ||p_n - g_m||^2 to ~1e-5."""
    u = (-2.0 * p.T).astype(np.float32)          # [3, N]
    v = np.ascontiguousarray(g.T)                # [3, M]
    p2 = (p * p).sum(1, dtype=np.float32)
    g2 = (g * g).sum(1, dtype=np.float32)
    uh, ul = _split16(u)
    vh, vl = _split16(v)
    p2h, p2l = _split16(p2)
    g2h, g2l = _split16(g2)
    onesN = np.ones(p.shape[0], np.float16)
    onesM = np.ones(g.shape[0], np.float16)
    A_rows, B_rows = [], []
    for d in range(D):
        A_rows += [uh[d], uh[d], ul[d]]
        B_rows += [vh[d], vl[d], vh[d]]
    A_rows += [p2h, p2l, onesN, onesN]
    B_rows += [onesM, onesM, g2h, g2l]
    return np.stack(A_rows), np.stack(B_rows)


def kernel(pred: np.ndarray, gt: np.ndarray) -> np.ndarray:
    pred = np.asarray(pred, dtype=np.float32)
    gt = np.asarray(gt, dtype=np.float32)
    assert pred.shape == (B, N, D) and gt.shape == (B, M, D)

    in_maps = []
    for b in range(B):
        A, Bm = _make_aug(pred[b], gt[b])
        in_maps.append({"a": A, "b": Bm})

    if "nc" not in _NC_CACHE:
        _NC_CACHE["nc"] = _build_nc()
    nc = _NC_CACHE["nc"]

    trace = bool(int(os.environ.get("KERNEL_TRACE", "0")))
    res = run_bass_kernel_spmd(nc, in_maps, _CORES, trace=trace)
    LAST_PROFILE.clear()
    LAST_PROFILE.update(
        exec_time_ns=res.exec_time_ns, mean_exec_time_ns=res.mean_exec_time_ns
    )
    if trace and res.instructions_and_trace is not None:
        LAST_PROFILE["trace_path"] = res.instructions_and_trace[1]

    total = 0.0
    for b in range(B):
        rs, cs = (float(x) for x in res.results[b]["out"][0])
        total += 0.5 * (rs / N + cs / M)
    return np.float32(total / B * 100.0)


# revision 9
# speedup vs baseline: 1.4714x; 1.0197x over previous
"""Chamfer-distance kernel for Trainium2 (nn_CD_1013612282415).

Full inputs: pred [8, 8192, 3] f32, gt [8, 8192, 3] f32.
Output: scalar f32 = mean_b(0.5*mean_n min_m ||p-g||^2 + 0.5*mean_m min_n) * 100.

Sharding: one batch element per NeuronCore (8 cores).

Per-core algorithm:
  The squared-distance matrix is computed on the PE as a single K=13 fp16
  matmul per tile: each operand value is hi/lo-split into two fp16s and the
  product u*v expanded as uh*vh + uh*vl + ul*vh across K-rows (K-rows are
  free: matmul cost is free-dim cycles only). This gives ~1e-5 abs accuracy
  (vs ~7e-3 for a plain fp16/bf16 matmul) at full bf16 streaming rate --
  4x faster than the native fp32 matmul path.

  dis tiles land in PSUM [128, 2048] f32. ScalarE casts them to bf16 in
  SBUF; VectorE then does the row-min via tensor_scalar's min-accumulator
  (single-source op -> 4x DVE mode) and folds the running col-min via
  bf16 tensor_tensor min (2x mode). The col-min partition reduction runs
  at the end via PE transposes + strided tensor_reduce. Per-core output is
  [sum_n rowmin, sum_m colmin]; the host combines 8 pairs in float64.
"""
import os
import sys

for _p in ("/opt/trn_rl_repo",):
    if _p not in sys.path:
        sys.path.insert(0, _p)

import numpy as np
import concourse.bass as bass
import concourse.mybir as mybir
from concourse.tile import TileContext
from concourse.bass_utils import run_bass_kernel_spmd

B, N, M, D = 8, 8192, 8192, 3
K = 13            # 3 coord dims x 3 split rows + 2 (|p|^2) + 2 (|g|^2)
PCHUNK = 128      # n rows per matmul tile (partition dim)
FD = 2048         # m columns per PSUM supertile (4 banks)
NI = N // PCHUNK  # 64 n-chunks
NJ = M // FD      # 4 m-superchunks
MM_N = 512        # columns per matmul (one PSUM bank)
BIG = 3.0e38

_CORES = list(range(8))
_NC_CACHE = {}
LAST_PROFILE = {}


def _split_waits(nc, max_waits=1):
    """This container's pinned walrus rejects >1 sync-wait per instruction;
    move excess waits onto InstNoOps inserted just before the offender."""
    for f in nc.m.functions:
        for bb in f.blocks:
            insts = list(bb.instructions)
            out, changed = [], False
            for inst in insts:
                si = inst.sync_info
                if si is not None and len(si.on_wait) > max_waits:
                    waits = list(si.on_wait)
                    extra, keep = waits[:-max_waits], waits[-max_waits:]
                    for i in range(0, len(extra), max_waits):
                        nop = mybir.InstNoOp(
                            name=f"{inst.name}-wsplit-{i}",
                            sync_info=mybir.SyncInfo(
                                on_wait=extra[i : i + max_waits], on_update=[]
                            ),
                        )
                        nop.engine = inst.engine
                        out.append(nop)
                    inst.sync_info = mybir.SyncInfo(
                        on_wait=keep, on_update=list(si.on_update)
                    )
                    changed = True
                out.append(inst)
            if changed:
                bb.instructions = out


def _build_nc():
    f16, f32, bf16, i32 = (
        mybir.dt.float16,
        mybir.dt.float32,
        mybir.dt.bfloat16,
        mybir.dt.int32,
    )
    nc = bass.Bass(trn_type="TRN2")
    a_dram = nc.declare_dram_parameter("a", [K, N], f16, isOutput=False)
    b_dram = nc.declare_dram_parameter("b", [K, M], f16, isOutput=False)
    out_dram = nc.declare_dram_parameter("out", [1, 2], f32, isOutput=True)

    with TileContext(nc) as tc:
        with (
            tc.tile_pool(name="io", bufs=1) as io,
            tc.tile_pool(name="work", bufs=1) as work,
            tc.tile_pool(name="dis", bufs=4) as disp,
            tc.tile_pool(name="rowt", bufs=4) as rowt,
        ):
            a_sb = io.tile([K, N], f16)
            b_sb = io.tile([K, M], f16)
            nc.sync.dma_start(out=a_sb[:], in_=a_dram.ap())
            nc.sync.dma_start(out=b_sb[:], in_=b_dram.ap())

            colmin = work.tile([PCHUNK, M], bf16, name="colmin")
            nc.vector.memset(colmin[:], BIG)
            rowmins = work.tile([PCHUNK, NI], f32)

            # identity (bf16) for PE transposes, built on device
            col_i = work.tile([PCHUNK, PCHUNK], i32)
            part_i = work.tile([PCHUNK, PCHUNK], i32)
            nc.gpsimd.iota(col_i[:], pattern=[[1, PCHUNK]], channel_multiplier=0)
            nc.gpsimd.iota(part_i[:], pattern=[[0, PCHUNK]], channel_multiplier=1)
            ident = work.tile([PCHUNK, PCHUNK], bf16)
            nc.vector.tensor_tensor(
                ident[:], col_i[:], part_i[:], mybir.AluOpType.is_equal
            )

            with tc.tile_pool(name="ps", bufs=2, space="PSUM") as ps:
                for i in range(NI):
                    lhsT = a_sb[:, i * PCHUNK : (i + 1) * PCHUNK]
                    # contiguous bf16 row of all NJ supertiles for wide DVE ops
                    drow = disp.tile([PCHUNK, M], bf16, name="drow", bufs=3)
                    for j in range(NJ):
                        psum = ps.tile([PCHUNK, FD], f32, name="psum")
                        for s in range(FD // MM_N):
                            c0 = j * FD + s * MM_N
                            nc.tensor.matmul(
                                psum[:, s * MM_N : (s + 1) * MM_N],
                                lhsT,
                                b_sb[:, c0 : c0 + MM_N],
                                start=True,
                                stop=True,
                            )
                        nc.scalar.copy(drow[:, j * FD : (j + 1) * FD], psum[:])
                    # one wide col-min fold: visits 2x8192 inputs at 4/cyc
                    nc.vector.tensor_tensor(
                        colmin[:], drow[:], colmin[:], mybir.AluOpType.min
                    )
                    # row-min: pairwise halving tree, then one 1x reduce
                    t1 = rowt.tile([PCHUNK, M // 2], bf16, name="t1", bufs=2)
                    nc.vector.tensor_tensor(
                        t1[:], drow[:, : M // 2], drow[:, M // 2 :], mybir.AluOpType.min
                    )
                    w = M // 4
                    while w >= 512:
                        nc.vector.tensor_tensor(
                            t1[:, :w], t1[:, :w], t1[:, w : 2 * w], mybir.AluOpType.min
                        )
                        w //= 2
                    nc.vector.tensor_reduce(
                        rowmins[:, i : i + 1],
                        t1[:, : 2 * w],
                        mybir.AxisListType.X,
                        mybir.AluOpType.min,
                    )

            # epilogue: col-min partition reduction via PE transposes
            sums = work.tile([PCHUNK, 2], f32)
            cmin_t = work.tile([PCHUNK, NJ * (FD // PCHUNK)], f32, name="cmin_t")
            with tc.tile_pool(name="pst", bufs=2, space="PSUM") as pst:
                for j in range(NJ):
                    tp = pst.tile([PCHUNK, FD], bf16, name="tp")
                    for k in range(FD // PCHUNK):
                        c0 = j * FD + k * PCHUNK
                        nc.tensor.transpose(
                            tp[:, k * PCHUNK : (k + 1) * PCHUNK],
                            colmin[:, c0 : c0 + PCHUNK],
                            ident[:],
                        )
                    nb = FD // PCHUNK  # 16 blocks
                    nc.vector.tensor_reduce(
                        cmin_t[:, j * nb : (j + 1) * nb],
                        tp[:].rearrange("p (k q) -> p k q", q=PCHUNK),
                        mybir.AxisListType.X,
                        mybir.AluOpType.min,
                    )
                nc.vector.tensor_reduce(
                    sums[:, 0:1], rowmins[:], mybir.AxisListType.X, mybir.AluOpType.add
                )
                nc.vector.tensor_reduce(
                    sums[:, 1:2], cmin_t[:], mybir.AxisListType.X, mybir.AluOpType.add
                )
                ones = work.tile([PCHUNK, 1], f32)
                nc.vector.memset(ones[:], 1.0)
                out_ps = pst.tile([1, 2], f32, name="out_ps")
                nc.tensor.matmul(out_ps[:], ones[:], sums[:], start=True, stop=True)
                out_sb = work.tile([1, 2], f32)
                nc.scalar.copy(out_sb[:], out_ps[:])
                nc.sync.dma_start(out=out_dram.ap(), in_=out_sb[:])

    _split_waits(nc)
    return nc


def _split16(x):
    hi = x.astype(np.float16)
    lo = (x.astype(np.float32) - hi.astype(np.float32)).astype(np.float16)
    return hi, lo


def _make_aug(p, g):
    """p [N,3] f32, g [M,3] f32 -> A [13, N] f16, B [13, M] f16 such that
    (A.T @ B)[n, m] ~= ||p_n - g_m||^2 to ~1e-5."""
    u = (-2.0 * p.T).astype(np.float32)          # [3, N]
    v = np.ascontiguousarray(g.T)                # [3, M]
    p2 = (p * p).sum(1, dtype=np.float32)
    g2 = (g * g).sum(1, dtype=np.float32)
    uh, ul = _split16(u)
    vh, vl = _split16(v)
    p2h, p2l = _split16(p2)
    g2h, g2l = _split16(g2)
    onesN = np.ones(p.shape[0], np.float16)
    onesM = np.ones(g.shape[0], np.float16)
    A_rows, B_rows = [], []
    for d in range(D):
        A_rows += [uh[d], uh[d], ul[d]]
        B_rows += [vh[d], vl[d], vh[d]]
    A_rows += [p2h, p2l, onesN, onesN]
    B_rows += [onesM, onesM, g2h, g2l]
    return np.stack(A_rows), np.stack(B_rows)


def kernel(pred: np.ndarray, gt: np.ndarray) -> np.ndarray:
    pred = np.asarray(pred, dtype=np.float32)
    gt = np.asarray(gt, dtype=np.float32)
    assert pred.shape == (B, N, D) and gt.shape == (B, M, D)

    in_maps = []
    for b in range(B):
        A, Bm = _make_aug(pred[b], gt[b])
        in_maps.append({"a": A, "b": Bm})

    if "nc" not in _NC_CACHE:
        _NC_CACHE["nc"] = _build_nc()
    nc = _NC_CACHE["nc"]

    trace = bool(int(os.environ.get("KERNEL_TRACE", "0")))
    res = run_bass_kernel_spmd(nc, in_maps, _CORES, trace=trace)
    LAST_PROFILE.clear()
    LAST_PROFILE.update(
        exec_time_ns=res.exec_time_ns, mean_exec_time_ns=res.mean_exec_time_ns
    )
    if trace and res.instructions_and_trace is not None:
        LAST_PROFILE["trace_path"] = res.instructions_and_trace[1]

    total = 0.0
    for b in range(B):
        rs, cs = (float(x) for x in res.results[b]["out"][0])
        total += 0.5 * (rs / N + cs / M)
    return np.float32(total / B * 100.0)


# revision 10
# speedup vs baseline: 1.4719x; 1.0003x over previous
"""Chamfer-distance kernel for Trainium2 (nn_CD_1013612282415).

Full inputs: pred [8, 8192, 3] f32, gt [8, 8192, 3] f32.
Output: scalar f32 = mean_b(0.5*mean_n min_m ||p-g||^2 + 0.5*mean_m min_n) * 100.

Sharding: one batch element per NeuronCore (8 cores).

Per-core algorithm:
  The squared-distance matrix is computed on the PE as a single K=13 fp16
  matmul per tile: each operand value is hi/lo-split into two fp16s and the
  product u*v expanded as uh*vh + uh*vl + ul*vh across K-rows (K-rows are
  free: matmul cost is free-dim cycles only). This gives ~1e-5 abs accuracy
  (vs ~7e-3 for a plain fp16/bf16 matmul) at full bf16 streaming rate --
  4x faster than the native fp32 matmul path.

  dis tiles land in PSUM [128, 2048] f32. ScalarE casts them to bf16 in
  SBUF; VectorE then does the row-min via tensor_scalar's min-accumulator
  (single-source op -> 4x DVE mode) and folds the running col-min via
  bf16 tensor_tensor min (2x mode). The col-min partition reduction runs
  at the end via PE transposes + strided tensor_reduce. Per-core output is
  [sum_n rowmin, sum_m colmin]; the host combines 8 pairs in float64.
"""
import os
import sys

for _p in ("/opt/trn_rl_repo",):
    if _p not in sys.path:
        sys.path.insert(0, _p)

import numpy as np
import concourse.bass as bass
import concourse.mybir as mybir
from concourse.tile import TileContext
from concourse.bass_utils import run_bass_kernel_spmd

B, N, M, D = 8, 8192, 8192, 3
K = 13            # 3 coord dims x 3 split rows + 2 (|p|^2) + 2 (|g|^2)
PCHUNK = 128      # n rows per matmul tile (partition dim)
FD = 2048         # m columns per PSUM supertile (4 banks)
NI = N // PCHUNK  # 64 n-chunks
NJ = M // FD      # 4 m-superchunks
MM_N = 512        # columns per matmul (one PSUM bank)
BIG = 60000.0  # > max squared distance (~40); fits fp16

_CORES = list(range(8))
_NC_CACHE = {}
LAST_PROFILE = {}


def _split_waits(nc, max_waits=1):
    """This container's pinned walrus rejects >1 sync-wait per instruction;
    move excess waits onto InstNoOps inserted just before the offender."""
    for f in nc.m.functions:
        for bb in f.blocks:
            insts = list(bb.instructions)
            out, changed = [], False
            for inst in insts:
                si = inst.sync_info
                if si is not None and len(si.on_wait) > max_waits:
                    waits = list(si.on_wait)
                    extra, keep = waits[:-max_waits], waits[-max_waits:]
                    for i in range(0, len(extra), max_waits):
                        nop = mybir.InstNoOp(
                            name=f"{inst.name}-wsplit-{i}",
                            sync_info=mybir.SyncInfo(
                                on_wait=extra[i : i + max_waits], on_update=[]
                            ),
                        )
                        nop.engine = inst.engine
                        out.append(nop)
                    inst.sync_info = mybir.SyncInfo(
                        on_wait=keep, on_update=list(si.on_update)
                    )
                    changed = True
                out.append(inst)
            if changed:
                bb.instructions = out


def _build_nc():
    f16, f32, bf16, i32 = (
        mybir.dt.float16,
        mybir.dt.float32,
        mybir.dt.bfloat16,
        mybir.dt.int32,
    )
    nc = bass.Bass(trn_type="TRN2")
    a_dram = nc.declare_dram_parameter("a", [K, N], f16, isOutput=False)
    b_dram = nc.declare_dram_parameter("b", [K, M], f16, isOutput=False)
    out_dram = nc.declare_dram_parameter("out", [1, 2], f32, isOutput=True)

    with TileContext(nc) as tc:
        with (
            tc.tile_pool(name="io", bufs=1) as io,
            tc.tile_pool(name="work", bufs=1) as work,
            tc.tile_pool(name="dis", bufs=4) as disp,
            tc.tile_pool(name="rowt", bufs=4) as rowt,
        ):
            a_sb = io.tile([K, N], f16)
            b_sb = io.tile([K, M], f16)
            nc.sync.dma_start(out=a_sb[:], in_=a_dram.ap())
            nc.sync.dma_start(out=b_sb[:], in_=b_dram.ap())

            colmin = work.tile([PCHUNK, M], f16, name="colmin")
            nc.vector.memset(colmin[:], BIG)
            rowmins = work.tile([PCHUNK, NI], f32)

            # identity (bf16) for PE transposes, built on device
            col_i = work.tile([PCHUNK, PCHUNK], i32)
            part_i = work.tile([PCHUNK, PCHUNK], i32)
            nc.gpsimd.iota(col_i[:], pattern=[[1, PCHUNK]], channel_multiplier=0)
            nc.gpsimd.iota(part_i[:], pattern=[[0, PCHUNK]], channel_multiplier=1)
            ident = work.tile([PCHUNK, PCHUNK], f16)
            nc.vector.tensor_tensor(
                ident[:], col_i[:], part_i[:], mybir.AluOpType.is_equal
            )

            with tc.tile_pool(name="ps", bufs=2, space="PSUM") as ps:
                for i in range(NI):
                    lhsT = a_sb[:, i * PCHUNK : (i + 1) * PCHUNK]
                    # contiguous bf16 row of all NJ supertiles for wide DVE ops
                    drow = disp.tile([PCHUNK, M], f16, name="drow", bufs=3)
                    for j in range(NJ):
                        psum = ps.tile([PCHUNK, FD], f32, name="psum")
                        for s in range(FD // MM_N):
                            c0 = j * FD + s * MM_N
                            nc.tensor.matmul(
                                psum[:, s * MM_N : (s + 1) * MM_N],
                                lhsT,
                                b_sb[:, c0 : c0 + MM_N],
                                start=True,
                                stop=True,
                            )
                        nc.scalar.copy(drow[:, j * FD : (j + 1) * FD], psum[:])
                    # one wide col-min fold: visits 2x8192 inputs at 4/cyc
                    nc.vector.tensor_tensor(
                        colmin[:], drow[:], colmin[:], mybir.AluOpType.min
                    )
                    # row-min: pairwise halving tree, then one 1x reduce
                    t1 = rowt.tile([PCHUNK, M // 2], f16, name="t1", bufs=2)
                    nc.vector.tensor_tensor(
                        t1[:], drow[:, : M // 2], drow[:, M // 2 :], mybir.AluOpType.min
                    )
                    w = M // 4
                    while w >= 512:
                        nc.vector.tensor_tensor(
                            t1[:, :w], t1[:, :w], t1[:, w : 2 * w], mybir.AluOpType.min
                        )
                        w //= 2
                    nc.vector.tensor_reduce(
                        rowmins[:, i : i + 1],
                        t1[:, : 2 * w],
                        mybir.AxisListType.X,
                        mybir.AluOpType.min,
                    )

            # epilogue: col-min partition reduction via PE transposes
            sums = work.tile([PCHUNK, 2], f32)
            cmin_t = work.tile([PCHUNK, NJ * (FD // PCHUNK)], f32, name="cmin_t")
            with tc.tile_pool(name="pst", bufs=2, space="PSUM") as pst:
                for j in range(NJ):
                    tp = pst.tile([PCHUNK, FD], f16, name="tp")
                    for k in range(FD // PCHUNK):
                        c0 = j * FD + k * PCHUNK
                        nc.tensor.transpose(
                            tp[:, k * PCHUNK : (k + 1) * PCHUNK],
                            colmin[:, c0 : c0 + PCHUNK],
                            ident[:],
                        )
                    nb = FD // PCHUNK  # 16 blocks
                    nc.vector.tensor_reduce(
                        cmin_t[:, j * nb : (j + 1) * nb],
                        tp[:].rearrange("p (k q) -> p k q", q=PCHUNK),
                        mybir.AxisListType.X,
                        mybir.AluOpType.min,
                    )
                nc.vector.tensor_reduce(
                    sums[:, 0:1], rowmins[:], mybir.AxisListType.X, mybir.AluOpType.add
                )
                nc.vector.tensor_reduce(
                    sums[:, 1:2], cmin_t[:], mybir.AxisListType.X, mybir.AluOpType.add
                )
                ones = work.tile([PCHUNK, 1], f32)
                nc.vector.memset(ones[:], 1.0)
                out_ps = pst.tile([1, 2], f32, name="out_ps")
                nc.tensor.matmul(out_ps[:], ones[:], sums[:], start=True, stop=True)
                out_sb = work.tile([1, 2], f32)
                nc.scalar.copy(out_sb[:], out_ps[:])
                nc.sync.dma_start(out=out_dram.ap(), in_=out_sb[:])

    _split_waits(nc)
    return nc


def _split16(x):
    hi = x.astype(np.float16)
    lo = (x.astype(np.float32) - hi.astype(np.float32)).astype(np.float16)
    return hi, lo


def _make_aug(p, g):
    """p [N,3] f32, g [M,3] f32 -> A [13, N] f16, B [13, M] f16 such that
    (A.T @ B)[n, m] ~= ||p_n - g_m||^2 to ~1e-5."""
    u = (-2.0 * p.T).astype(np.float32)          # [3, N]
    v = np.ascontiguousarray(g.T)                # [3, M]
    p2 = (p * p).sum(1, dtype=np.float32)
    g2 = (g * g).sum(1, dtype=np.float32)
    uh, ul = _split16(u)
    vh, vl = _split16(v)
    p2h, p2l = _split16(p2)
    g2h, g2l = _split16(g2)
    onesN = np.ones(p.shape[0], np.float16)
    onesM = np.ones(g.shape[0], np.float16)
    A_rows, B_rows = [], []
    for d in range(D):
        A_rows += [uh[d], uh[d], ul[d]]
        B_rows += [vh[d], vl[d], vh[d]]
    A_rows += [p2h, p2l, onesN, onesN]
    B_rows += [onesM, onesM, g2h, g2l]
    return np.stack(A_rows), np.stack(B_rows)


def kernel(pred: np.ndarray, gt: np.ndarray) -> np.ndarray:
    pred = np.asarray(pred, dtype=np.float32)
    gt = np.asarray(gt, dtype=np.float32)
    assert pred.shape == (B, N, D) and gt.shape == (B, M, D)

    in_maps = []
    for b in range(B):
        A, Bm = _make_aug(pred[b], gt[b])
        in_maps.append({"a": A, "b": Bm})

    if "nc" not in _NC_CACHE:
        _NC_CACHE["nc"] = _build_nc()
    nc = _NC_CACHE["nc"]

    trace = bool(int(os.environ.get("KERNEL_TRACE", "0")))
    res = run_bass_kernel_spmd(nc, in_maps, _CORES, trace=trace)
    LAST_PROFILE.clear()
    LAST_PROFILE.update(
        exec_time_ns=res.exec_time_ns, mean_exec_time_ns=res.mean_exec_time_ns
    )
    if trace and res.instructions_and_trace is not None:
        LAST_PROFILE["trace_path"] = res.instructions_and_trace[1]

    total = 0.0
    for b in range(B):
        rs, cs = (float(x) for x in res.results[b]["out"][0])
        total += 0.5 * (rs / N + cs / M)
    return np.float32(total / B * 100.0)
